# revision 1
# baseline (speedup 1.0000x reference)
"""Trainium2 Bass kernel for nn_Custom_Loss_84937273246180.

reference:
    path = argmax(solution_matrix, axis=0)        # [8192] int
    nxt  = roll(path, -1)
    out  = sum(cost_matrix[path, nxt])            # [1] f32

Strategy (8 NeuronCores):
  Launch A (8 cores, column-sharded): core i owns columns [i*1024, (i+1)*1024)
    of solution_matrix and computes the full argmax over the 8192 rows for its
    columns. Row-fold via DVE max tree, cross-partition max via gpsimd
    partition_all_reduce, index extraction via is_equal mask + PE matmul with
    (p, t) index vectors. Outputs path_shard [1024] int32 per core.
    Host concatenates the 8 shards (pure unshard).
  Launch B (1 core): takes full path [8192] + cost_matrix, computes the
    rolled gather cost_matrix[path, nxt] via indirect DMA (gathers 256B
    chunks, then on-chip selection) and reduces to the scalar output.
"""

import os
import numpy as np
from contextlib import ExitStack

import concourse.bass as bass
import concourse.bacc as bacc
import concourse.tile as tile
from concourse import mybir
from concourse import bass_isa
from concourse.bass_utils import run_bass_kernel_spmd

N = 8192
NCORES = 8
CPC = N // NCORES        # columns per core = 1024
CHUNK = 512              # columns processed per chunk
NCHUNK = CPC // CHUNK    # 2
NTILES = N // 128        # 64 row tiles
NSUB = 8                 # sub-tiles (each holds 8 row tiles)

F32 = mybir.dt.float32
BF16 = mybir.dt.bfloat16
I32 = mybir.dt.int32

_cache = {}


def _build_argmax_nc(n_iters: int = 1):
    """SPMD kernel: input sol [NCHUNK, 8192, CHUNK] (one column shard,
    chunk-major so every DMA is a fully contiguous block), output
    path_shard [1024] int32 = argmax over axis 0 (rows).

    n_iters > 1 wraps the body in a For_i hardware loop (used only for
    timing; instruction count stays constant)."""
    nc = bacc.Bacc("TRN2", target_bir_lowering=False, debug=False,
                   num_devices=NCORES)
    sol = nc.dram_tensor("sol", [NCHUNK, N, CHUNK], F32, kind="ExternalInput")
    path_out = nc.dram_tensor("path_shard", [CPC], I32, kind="ExternalOutput")

    # [chunk, sub, p, a, c]: row = (s*NSUB + a)*128 + p
    sol_v = sol.rearrange("h (s a p) c -> h s p a c", p=128, a=NSUB)

    with tile.TileContext(nc) as tc:
        with ExitStack() as ctx:
            data_pool = ctx.enter_context(tc.tile_pool(name="data", bufs=NSUB))
            scratch_pool = ctx.enter_context(tc.tile_pool(name="scratch", bufs=1))
            mask_pool = ctx.enter_context(tc.tile_pool(name="mask", bufs=3))
            acc_pool = ctx.enter_context(tc.tile_pool(name="acc", bufs=1))
            small_pool = ctx.enter_context(tc.tile_pool(name="small", bufs=2))
            const_pool = ctx.enter_context(tc.tile_pool(name="const", bufs=1))
            psum_pool = ctx.enter_context(
                tc.tile_pool(name="psum", bufs=2, space="PSUM"))

            # lhsT index vectors: [128, t, 0] = p, [128, t, 1] = 128*t
            lt_i = const_pool.tile([128, NTILES, 2], I32)
            nc.gpsimd.iota(lt_i[:, :, 0], pattern=[[0, NTILES]], base=0,
                           channel_multiplier=1)
            nc.gpsimd.iota(lt_i[:, :, 1], pattern=[[128, NTILES]], base=0,
                           channel_multiplier=0)
            lt = const_pool.tile([128, NTILES, 2], BF16)
            nc.vector.tensor_copy(lt[:], lt_i[:])

            import contextlib
            loop_cm = (tc.For_i(0, n_iters, 1) if n_iters > 1
                       else contextlib.nullcontext())
            with loop_cm:
                for chunk in range(NCHUNK):
                    subs = []
                    msub = []
                    for s in range(NSUB):
                        st = data_pool.tile([128, NSUB, CHUNK], F32, tag="sub")
                        nc.sync.dma_start(out=st[:], in_=sol_v[chunk, s])
                        subs.append(st)
                        # per-sub-tile fold: 8 row tiles -> 1 (tree, no chain)
                        t1 = scratch_pool.tile([128, 4, CHUNK], F32, tag="t1")
                        nc.vector.tensor_tensor(
                            out=t1[:], in0=st[:, 0:4, :], in1=st[:, 4:8, :],
                            op=mybir.AluOpType.max)
                        t2 = scratch_pool.tile([128, 2, CHUNK], F32, tag="t2")
                        nc.vector.tensor_tensor(
                            out=t2[:], in0=t1[:, 0:2, :], in1=t1[:, 2:4, :],
                            op=mybir.AluOpType.max)
                        ms = acc_pool.tile([128, NSUB, CHUNK], F32, tag="ms")
                        if s == 0:
                            msub = [ms]
                        else:
                            ms = msub[0]
                        nc.vector.tensor_tensor(
                            out=ms[:, s, :], in0=t2[:, 0, :], in1=t2[:, 1, :],
                            op=mybir.AluOpType.max)
                    ms = msub[0]
                    # tree over the 8 sub-maxes
                    u1 = scratch_pool.tile([128, 4, CHUNK], F32, tag="t1")
                    nc.vector.tensor_tensor(
                        out=u1[:], in0=ms[:, 0:4, :], in1=ms[:, 4:8, :],
                        op=mybir.AluOpType.max)
                    u2 = scratch_pool.tile([128, 2, CHUNK], F32, tag="t2")
                    nc.vector.tensor_tensor(
                        out=u2[:], in0=u1[:, 0:2, :], in1=u1[:, 2:4, :],
                        op=mybir.AluOpType.max)
                    u3 = scratch_pool.tile([128, CHUNK], F32, tag="u3")
                    nc.vector.tensor_tensor(
                        out=u3[:], in0=u2[:, 0, :], in1=u2[:, 1, :],
                        op=mybir.AluOpType.max)

                    # column max replicated across partitions
                    bmax = small_pool.tile([128, CHUNK], F32, tag="bmax")
                    nc.gpsimd.partition_all_reduce(
                        bmax[:], u3[:], channels=128,
                        reduce_op=bass_isa.ReduceOp.max)

                    # cmp + index matmuls
                    ps = psum_pool.tile([2, CHUNK], F32, tag="ps")
                    for s in range(NSUB):
                        st = subs[s]
                        mask = mask_pool.tile([128, NSUB, CHUNK], BF16, tag="mask")
                        bmax_b = bass.AP(
                            tensor=bmax.tensor,
                            offset=bmax[:].offset,
                            ap=[bmax[:].ap[0], [0, NSUB], bmax[:].ap[1]],
                        )
                        nc.vector.tensor_tensor(
                            out=mask[:], in0=st[:], in1=bmax_b,
                            op=mybir.AluOpType.is_equal)
                        for j in range(NSUB):
                            t = s * NSUB + j
                            nc.tensor.matmul(
                                ps[:], lt[:, t, :], mask[:, j, :],
                                start=(t == 0), stop=(t == NTILES - 1))

                    # row = (128*t) + p ; psum row0 = sum p*mask, row1 = sum 128t*mask
                    sb2 = small_pool.tile([2, CHUNK], F32, tag="sb2")
                    nc.vector.tensor_copy(sb2[:], ps[:])
                    sbt = small_pool.tile([1, CHUNK], F32, tag="sbt")
                    nc.sync.dma_start(out=sbt[:], in_=sb2[1:2, :])
                    rowf = small_pool.tile([1, CHUNK], F32, tag="rowf")
                    nc.vector.tensor_tensor(
                        out=rowf[:], in0=sbt[:], in1=sb2[0:1, :],
                        op=mybir.AluOpType.add)
                    nc.vector.tensor_scalar(
                        out=rowf[:], in0=rowf[:], scalar1=float(N - 1),
                        scalar2=0.0, op0=mybir.AluOpType.min,
                        op1=mybir.AluOpType.max)
                    rowi = small_pool.tile([1, CHUNK], I32, tag="rowi")
                    nc.vector.tensor_copy(rowi[:], rowf[:])
                    nc.sync.dma_start(
                        out=path_out[chunk * CHUNK:(chunk + 1) * CHUNK],
                        in_=rowi[0:1, :])

    nc.compile()
    return nc


def _get_argmax_nc(n_iters: int = 1):
    key = ("argmax", n_iters)
    if key not in _cache:
        _cache[key] = _build_argmax_nc(n_iters)
    return _cache[key]


def run_argmax(solution_matrix: np.ndarray, n_iters: int = 1) -> np.ndarray:
    nc = _get_argmax_nc(n_iters)
    sol = np.ascontiguousarray(solution_matrix)
    in_maps = []
    for i in range(NCORES):
        shard = sol[:, i * CPC:(i + 1) * CPC]           # [8192, 1024]
        shard = shard.reshape(N, NCHUNK, CHUNK)          # [8192, 2, 512]
        shard = np.ascontiguousarray(shard.transpose(1, 0, 2))  # [2, 8192, 512]
        in_maps.append({"sol": shard})
    res = run_bass_kernel_spmd(nc, in_maps, core_ids=list(range(NCORES)))
    path = np.concatenate([res.results[i]["path_shard"] for i in range(NCORES)])
    return path.astype(np.int32)


def kernel(solution_matrix: np.ndarray, cost_matrix: np.ndarray) -> np.ndarray:
    path = run_argmax(solution_matrix)
    cost = run_gather(cost_matrix, path)
    return cost


# ---------------- Launch B: gather + sum ----------------

def _build_gather_nc(n_iters: int = 1):
    """Single-core kernel: inputs cost [8192, 8192] f32 + path [8192] i32;
    output out [1] f32 = sum_i cost[path[i], path[(i+1) % n]].

    Terms are laid out [128, G] with i = p*G + g.  flat = (path<<13)|nxt is
    exact int32; each of the G indirect DMAs gathers one f32 per partition
    (offset AP [128, 1] -- the hardware-verified form)."""
    G = N // 128  # 64 terms per partition
    nc = bacc.Bacc("TRN2", target_bir_lowering=False, debug=False,
                   num_devices=1)
    cost = nc.dram_tensor("cost", [N, N], F32, kind="ExternalInput")
    path_in = nc.dram_tensor("path", [N], I32, kind="ExternalInput")
    out = nc.dram_tensor("out", [1], F32, kind="ExternalOutput")

    cost1 = cost.rearrange("r (k e) -> (r k) e", e=1)  # [N*N, 1]

    with tile.TileContext(nc) as tc:
        with ExitStack() as ctx:
            pool = ctx.enter_context(tc.tile_pool(name="p", bufs=2))

            import contextlib
            loop_cm = (tc.For_i(0, n_iters, 1) if n_iters > 1
                       else contextlib.nullcontext())
            with loop_cm:
                pt = pool.tile([128, G], I32, tag="pt")
                nc.sync.dma_start(
                    out=pt[:], in_=path_in.rearrange("(p g) -> p g", g=G))
                nx = pool.tile([128, G], I32, tag="nx")
                nc.sync.dma_start(
                    out=nx[0:127, :],
                    in_=path_in[1:1 + 127 * G].rearrange("(p g) -> p g", g=G))
                nc.sync.dma_start(
                    out=nx[127:128, 0:G - 1],
                    in_=path_in[1 + 127 * G:N].rearrange("(o x) -> o x", o=1))
                nc.sync.dma_start(
                    out=nx[127:128, G - 1:G],
                    in_=path_in[0:1].rearrange("(o x) -> o x", o=1))

                # flat = (pt << 13) | nx  (exact int32 bitwise)
                fl = pool.tile([128, G], I32, tag="fl")
                nc.vector.tensor_scalar(
                    out=fl[:], in0=pt[:], scalar1=13, scalar2=None,
                    op0=mybir.AluOpType.logical_shift_left)
                nc.vector.tensor_tensor(
                    out=fl[:], in0=fl[:], in1=nx[:],
                    op=mybir.AluOpType.bitwise_or)

                vals = pool.tile([128, G], F32, tag="vals")
                for g in range(G):
                    nc.gpsimd.indirect_dma_start(
                        out=vals[:, g:g + 1], out_offset=None,
                        in_=cost1[:, :],
                        in_offset=bass.IndirectOffsetOnAxis(
                            ap=fl[:, g:g + 1], axis=0))

                s1 = pool.tile([128, 1], F32, tag="s1")
                nc.vector.reduce_sum(s1[:], vals[:], axis=mybir.AxisListType.X)
                s2 = pool.tile([128, 1], F32, tag="s2")
                nc.gpsimd.partition_all_reduce(
                    s2[:], s1[:], channels=128,
                    reduce_op=bass_isa.ReduceOp.add)
                nc.sync.dma_start(out=out[0:1], in_=s2[0:1, 0:1])

    nc.compile()
    return nc


def _get_gather_nc(n_iters: int = 1):
    key = ("gather", n_iters)
    if key not in _cache:
        _cache[key] = _build_gather_nc(n_iters)
    return _cache[key]


def run_gather(cost_matrix: np.ndarray, path: np.ndarray,
               n_iters: int = 1) -> np.ndarray:
    nc = _get_gather_nc(n_iters)
    res = run_bass_kernel_spmd(
        nc,
        [{"cost": np.ascontiguousarray(cost_matrix),
          "path": np.ascontiguousarray(path.astype(np.int32))}],
        core_ids=[0])
    return np.asarray(res.results[0]["out"], dtype=np.float32)


if __name__ == "__main__":
    rng = np.random.default_rng(0)
    sol = rng.standard_normal((N, N), dtype=np.float32)
    cm = rng.random((N, N), dtype=np.float32)
    path = run_argmax(sol)
    want = sol.argmax(axis=0)
    print("argmax match:", np.array_equal(path, want),
          (path != want).sum(), "mismatches")



# revision 2
# speedup vs baseline: 1.4126x; 1.4126x over previous
"""Trainium2 Bass kernel v2 for nn_Custom_Loss_84937273246180.

reference:
    path = argmax(solution_matrix, axis=0)        # [8192] int
    nxt  = roll(path, -1)
    out  = sum(cost_matrix[path, nxt])            # [1] f32

Strategy (8 NeuronCores, two launches):

Launch A (argmax, column-sharded): core k owns columns [1024k, 1024(k+1)).
  Host reshapes its [8192, 1024] shard to [8 groups, 128 p, 8192] where
  row = g*1024 + p*8 + a (pure row-major reshape; per-(g,p) DRAM run is a
  contiguous 32KB block -> 128 big descriptors per DMA).
  Per group: one 4MB DMA, DVE max-tree to the group column max, gpsimd
  partition_all_reduce to replicate it, is_equal mask * (row+1) and a max
  fold + partition_all_reduce to extract the argmax row, then a running
  cross-group combine on [1, 1024] slices.  No PE, no PSUM, ~15 instrs
  per group, folds done in place to fit SBUF.

Launch B (gather, term-sharded): every core gets the full cost matrix
  (viewed [1048576, 64]) plus a 1025-entry path segment (1024 terms + the
  wrap element, host-sliced).  Each core gathers 256B blocks containing
  its 1024 terms via 8 indirect DMAs (offset AP [128, 1], D=64 -- the
  fast, hardware-verified form), selects the in-block element with an
  iota/is_equal mask, reduces, and emits a [1] partial; host adds the 8
  partials.
"""

import contextlib
import numpy as np
from contextlib import ExitStack

import concourse.bass as bass
import concourse.bacc as bacc
import concourse.tile as tile
from concourse import mybir
from concourse import bass_isa
from concourse.bass_utils import run_bass_kernel_spmd

N = 8192
NCORES = 8
CPC = N // NCORES        # columns per core = 1024
NGRP = 8                 # row groups per core
NSUB = 8                 # sub-rows per partition per group
# row = g*1024 + p*8 + a

F32 = mybir.dt.float32
BF16 = mybir.dt.bfloat16
I32 = mybir.dt.int32

_cache = {}


def _bcast(ap_obj, insert_at, count):
    """Return a bass.AP equal to ap_obj with a stride-0 dim inserted."""
    dims = list(ap_obj.ap)
    dims.insert(insert_at, [0, count])
    return bass.AP(tensor=ap_obj.tensor, offset=ap_obj.offset, ap=dims)


# ---------------- Launch A: argmax ----------------

def _build_argmax_nc(n_iters: int = 1):
    nc = bacc.Bacc("TRN2", target_bir_lowering=False, debug=False,
                   num_devices=NCORES)
    sol = nc.dram_tensor("sol", [NGRP, 128, NSUB * CPC], F32,
                         kind="ExternalInput")
    path_out = nc.dram_tensor("path_shard", [CPC], I32, kind="ExternalOutput")

    with tile.TileContext(nc) as tc:
        with ExitStack() as ctx:
            data_pool = ctx.enter_context(tc.tile_pool(name="data", bufs=2))
            midx_pool = ctx.enter_context(tc.tile_pool(name="midx", bufs=1))
            l_pool = ctx.enter_context(tc.tile_pool(name="l", bufs=1))
            par_pool = ctx.enter_context(tc.tile_pool(name="par", bufs=2))
            run_pool = ctx.enter_context(tc.tile_pool(name="run", bufs=2))
            const_pool = ctx.enter_context(tc.tile_pool(name="const", bufs=1))
            out_pool = ctx.enter_context(tc.tile_pool(name="out", bufs=1))

            # idx0[p, a] = p*8 + a + 1  (row+1 base within a group)
            idx0_i = const_pool.tile([128, NSUB], I32)
            nc.gpsimd.iota(idx0_i[:], pattern=[[1, NSUB]], base=1,
                           channel_multiplier=NSUB)
            idx0 = const_pool.tile([128, NSUB], F32)
            nc.vector.tensor_copy(idx0[:], idx0_i[:])

            loop_cm = (tc.For_i(0, n_iters, 1) if n_iters > 1
                       else contextlib.nullcontext())
            with loop_cm:
                B = R = None
                for g in range(NGRP):
                    # per-group row+1 constants: idx0 + g*1024
                    idxg = const_pool.tile([128, NSUB], F32, tag="idxg")
                    nc.vector.tensor_scalar(
                        out=idxg[:], in0=idx0[:], scalar1=float(g * 1024),
                        scalar2=None, op0=mybir.AluOpType.add)

                    T = data_pool.tile([128, NSUB, CPC], F32, tag="T")
                    nc.sync.dma_start(out=T[:], in_=sol[g].rearrange(
                        "p (a c) -> p a c", a=NSUB))

                    # value tree: 8 -> 4 -> 2 -> 1 (L1 then in-place)
                    L1 = l_pool.tile([128, 4, CPC], F32, tag="L1")
                    nc.vector.tensor_tensor(
                        out=L1[:], in0=T[:, 0:4, :], in1=T[:, 4:8, :],
                        op=mybir.AluOpType.max)
                    nc.vector.tensor_tensor(
                        out=L1[:, 0:2, :], in0=L1[:, 0:2, :],
                        in1=L1[:, 2:4, :], op=mybir.AluOpType.max)
                    nc.vector.tensor_tensor(
                        out=L1[:, 0, :], in0=L1[:, 0, :], in1=L1[:, 1, :],
                        op=mybir.AluOpType.max)

                    Bg = par_pool.tile([128, CPC], F32, tag="Bg")
                    nc.gpsimd.partition_all_reduce(
                        Bg[:], L1[:, 0, :], channels=128,
                        reduce_op=bass_isa.ReduceOp.max)

                    # masked row+1: midx = (T == Bg) * idxg, then fold max
                    midx = midx_pool.tile([128, NSUB, CPC], F32, tag="midx")
                    nc.vector.tensor_tensor(
                        out=midx[:], in0=T[:], in1=_bcast(Bg[:], 1, NSUB),
                        op=mybir.AluOpType.is_equal)
                    nc.vector.tensor_tensor(
                        out=midx[:], in0=midx[:], in1=_bcast(idxg[:], 2, CPC),
                        op=mybir.AluOpType.mult)
                    nc.vector.tensor_tensor(
                        out=midx[:, 0:4, :], in0=midx[:, 0:4, :],
                        in1=midx[:, 4:8, :], op=mybir.AluOpType.max)
                    nc.vector.tensor_tensor(
                        out=midx[:, 0:2, :], in0=midx[:, 0:2, :],
                        in1=midx[:, 2:4, :], op=mybir.AluOpType.max)
                    nc.vector.tensor_tensor(
                        out=midx[:, 0, :], in0=midx[:, 0, :],
                        in1=midx[:, 1, :], op=mybir.AluOpType.max)

                    Rg = par_pool.tile([128, CPC], F32, tag="Rg")
                    nc.gpsimd.partition_all_reduce(
                        Rg[:], midx[:, 0, :], channels=128,
                        reduce_op=bass_isa.ReduceOp.max)

                    # running combine on [1, CPC] slices
                    if g == 0:
                        B, R = Bg, Rg
                    else:
                        Bn = run_pool.tile([1, CPC], F32, tag="Bn")
                        nc.vector.tensor_tensor(
                            out=Bn[:], in0=B[0:1, :], in1=Bg[0:1, :],
                            op=mybir.AluOpType.max)
                        a1 = run_pool.tile([1, CPC], F32, tag="a1")
                        nc.vector.tensor_tensor(
                            out=a1[:], in0=B[0:1, :], in1=Bn[:],
                            op=mybir.AluOpType.is_equal)
                        nc.vector.tensor_tensor(
                            out=a1[:], in0=a1[:], in1=R[0:1, :],
                            op=mybir.AluOpType.mult)
                        a2 = run_pool.tile([1, CPC], F32, tag="a2")
                        nc.vector.tensor_tensor(
                            out=a2[:], in0=Bg[0:1, :], in1=Bn[:],
                            op=mybir.AluOpType.is_equal)
                        nc.vector.tensor_tensor(
                            out=a2[:], in0=a2[:], in1=Rg[0:1, :],
                            op=mybir.AluOpType.mult)
                        Rn = run_pool.tile([1, CPC], F32, tag="Rn")
                        nc.vector.tensor_tensor(
                            out=Rn[:], in0=a1[:], in1=a2[:],
                            op=mybir.AluOpType.max)
                        B, R = Bn, Rn

                # path = R - 1 -> int32
                pf = out_pool.tile([1, CPC], F32, tag="pf")
                nc.vector.tensor_scalar(
                    out=pf[:], in0=R[0:1, :] if R.shape[0] != 1 else R[:],
                    scalar1=-1.0, scalar2=None, op0=mybir.AluOpType.add)
                pi = out_pool.tile([1, CPC], I32, tag="pi")
                nc.vector.tensor_copy(pi[:], pf[:])
                nc.sync.dma_start(out=path_out[0:CPC], in_=pi[0:1, :])

    nc.compile()
    return nc


def _get_argmax_nc(n_iters: int = 1):
    key = ("argmax", n_iters)
    if key not in _cache:
        _cache[key] = _build_argmax_nc(n_iters)
    return _cache[key]


def run_argmax(solution_matrix: np.ndarray, n_iters: int = 1) -> np.ndarray:
    nc = _get_argmax_nc(n_iters)
    in_maps = []
    for k in range(NCORES):
        shard = np.ascontiguousarray(
            solution_matrix[:, k * CPC:(k + 1) * CPC])
        # [8192, 1024] -> [8, 128, 8*1024]; row = g*1024 + p*8 + a
        shard = shard.reshape(NGRP, 128, NSUB * CPC)
        in_maps.append({"sol": shard})
    res = run_bass_kernel_spmd(nc, in_maps, core_ids=list(range(NCORES)))
    path = np.concatenate([res.results[k]["path_shard"]
                           for k in range(NCORES)])
    return path.astype(np.int32)


# ---------------- Launch B: gather + sum ----------------

def _build_gather_nc(n_iters: int = 1):
    """SPMD: inputs cost [N*N/64, 64] f32 + pseg [1025] i32 (this core's
    1024 terms and the wrap element); output part [1] f32 =
    sum_j cost[pseg[j], pseg[j+1]] over the core's 1024 terms."""
    G = CPC // 128  # 8 terms per partition
    nc = bacc.Bacc("TRN2", target_bir_lowering=False, debug=False,
                   num_devices=NCORES)
    cost = nc.dram_tensor("cost", [N * N // 64, 64], F32,
                          kind="ExternalInput")
    pseg = nc.dram_tensor("pseg", [CPC + 1], I32, kind="ExternalInput")
    out = nc.dram_tensor("part", [1], F32, kind="ExternalOutput")

    with tile.TileContext(nc) as tc:
        with ExitStack() as ctx:
            pool = ctx.enter_context(tc.tile_pool(name="p", bufs=2))
            const_pool = ctx.enter_context(tc.tile_pool(name="c", bufs=1))

            # iota64[p, c] = c  (same in every partition)
            io64_i = const_pool.tile([128, 64], I32)
            nc.gpsimd.iota(io64_i[:], pattern=[[1, 64]], base=0,
                           channel_multiplier=0)
            io64 = const_pool.tile([128, 64], F32)
            nc.vector.tensor_copy(io64[:], io64_i[:])

            loop_cm = (tc.For_i(0, n_iters, 1) if n_iters > 1
                       else contextlib.nullcontext())
            with loop_cm:
                # ptx[p, j] = pseg[p*G + j], j in [0, G]  (overlapping rows)
                ptx = pool.tile([128, G + 1], I32, tag="ptx")
                full = pseg[:]
                src = bass.AP(tensor=full.tensor, offset=full.offset,
                              ap=[[G, 128], [1, G + 1]])
                nc.sync.dma_start(out=ptx[:], in_=src)

                # flat = (pt << 13) | nx ; block = flat >> 6 ; rem = flat & 63
                fl = pool.tile([128, G], I32, tag="fl")
                nc.vector.tensor_scalar(
                    out=fl[:], in0=ptx[:, 0:G], scalar1=13, scalar2=None,
                    op0=mybir.AluOpType.logical_shift_left)
                nc.vector.tensor_tensor(
                    out=fl[:], in0=fl[:], in1=ptx[:, 1:G + 1],
                    op=mybir.AluOpType.bitwise_or)
                blk = pool.tile([128, G], I32, tag="blk")
                nc.vector.tensor_scalar(
                    out=blk[:], in0=fl[:], scalar1=6, scalar2=None,
                    op0=mybir.AluOpType.logical_shift_right)
                remi = pool.tile([128, G], I32, tag="remi")
                nc.vector.tensor_scalar(
                    out=remi[:], in0=fl[:], scalar1=63, scalar2=None,
                    op0=mybir.AluOpType.bitwise_and)
                rem = pool.tile([128, G], F32, tag="rem")
                nc.vector.tensor_copy(rem[:], remi[:])

                vals = pool.tile([128, G, 64], F32, tag="vals")
                for g in range(G):
                    nc.gpsimd.indirect_dma_start(
                        out=vals[:, g, :], out_offset=None,
                        in_=cost[:, :],
                        in_offset=bass.IndirectOffsetOnAxis(
                            ap=blk[:, g:g + 1], axis=0))

                # mask[p,g,c] = (io64[p,c] == rem[p,g]) ; dot with vals
                mask = pool.tile([128, G, 64], F32, tag="mask")
                nc.vector.tensor_tensor(
                    out=mask[:], in0=_bcast(io64[:], 1, G),
                    in1=_bcast(rem[:], 2, 64), op=mybir.AluOpType.is_equal)
                nc.vector.tensor_tensor(
                    out=mask[:], in0=mask[:], in1=vals[:],
                    op=mybir.AluOpType.mult)
                s1 = pool.tile([128, 1], F32, tag="s1")
                nc.vector.reduce_sum(
                    s1[:], mask[:].rearrange("p g c -> p (g c)"),
                    axis=mybir.AxisListType.X)
                s2 = pool.tile([128, 1], F32, tag="s2")
                nc.gpsimd.partition_all_reduce(
                    s2[:], s1[:], channels=128,
                    reduce_op=bass_isa.ReduceOp.add)
                nc.sync.dma_start(out=out[0:1], in_=s2[0:1, 0:1])

    nc.compile()
    return nc


def _get_gather_nc(n_iters: int = 1):
    key = ("gather", n_iters)
    if key not in _cache:
        _cache[key] = _build_gather_nc(n_iters)
    return _cache[key]


def run_gather(cost_matrix: np.ndarray, path: np.ndarray,
               n_iters: int = 1) -> np.ndarray:
    nc = _get_gather_nc(n_iters)
    cost_v = np.ascontiguousarray(cost_matrix).reshape(N * N // 64, 64)
    pfull = np.concatenate([path.astype(np.int32), path[:1].astype(np.int32)])
    in_maps = []
    for k in range(NCORES):
        in_maps.append({
            "cost": cost_v,
            "pseg": np.ascontiguousarray(pfull[k * CPC:(k + 1) * CPC + 1]),
        })
    res = run_bass_kernel_spmd(nc, in_maps, core_ids=list(range(NCORES)))
    total = np.float32(0.0)
    for k in range(NCORES):
        total += np.asarray(res.results[k]["part"], dtype=np.float32)[0]
    return np.asarray([total], dtype=np.float32)


def kernel(solution_matrix: np.ndarray, cost_matrix: np.ndarray) -> np.ndarray:
    path = run_argmax(solution_matrix)
    return run_gather(cost_matrix, path)


if __name__ == "__main__":
    rng = np.random.default_rng(0)
    sol = rng.standard_normal((N, N), dtype=np.float32)
    cm = rng.random((N, N), dtype=np.float32)
    path = run_argmax(sol)
    want = sol.argmax(axis=0)
    print("argmax match:", np.array_equal(path, want),
          int((path != want).sum()), "mismatches")
    got = run_gather(cm, path)
    nxt = np.roll(want, -1)
    exp = cm[want, nxt].sum()
    print("cost:", got, "expected:", exp,
          "rel:", abs(got[0] - exp) / abs(exp))


# revision 5
# speedup vs baseline: 20.1220x; 14.2448x over previous
"""Trainium2 Bass kernel v2 for nn_Custom_Loss_84937273246180.

reference:
    path = argmax(solution_matrix, axis=0)        # [8192] int
    nxt  = roll(path, -1)
    out  = sum(cost_matrix[path, nxt])            # [1] f32

Strategy (8 NeuronCores, two launches):

Launch A (argmax, column-sharded): core k owns columns [1024k, 1024(k+1)).
  Host reshapes its [8192, 1024] shard to [8 groups, 128 p, 8192] where
  row = g*1024 + p*8 + a (pure row-major reshape; per-(g,p) DRAM run is a
  contiguous 32KB block -> 128 big descriptors per DMA).
  Per group: one 4MB DMA, DVE max-tree to the group column max, gpsimd
  partition_all_reduce to replicate it, is_equal mask * (row+1) and a max
  fold + partition_all_reduce to extract the argmax row, then a running
  cross-group combine on [1, 1024] slices.  No PE, no PSUM, ~15 instrs
  per group, folds done in place to fit SBUF.

Launch B (gather, term-sharded): every core gets the full cost matrix
  (viewed [1048576, 64]) plus a 1025-entry path segment (1024 terms + the
  wrap element, host-sliced).  Each core gathers 256B blocks containing
  its 1024 terms via 8 indirect DMAs (offset AP [128, 1], D=64 -- the
  fast, hardware-verified form), selects the in-block element with an
  iota/is_equal mask, reduces, and emits a [1] partial; host adds the 8
  partials.
"""

import contextlib
import numpy as np
from contextlib import ExitStack

import concourse.bass as bass
import concourse.bacc as bacc
import concourse.tile as tile
from concourse import mybir
from concourse import bass_isa
from concourse.bass_utils import run_bass_kernel_spmd

N = 8192
NCORES = 8
CPC = N // NCORES        # columns per core = 1024
NGRP = 8                 # row groups per core
NSUB = 8                 # sub-rows per partition per group
# row = g*1024 + p*8 + a

F32 = mybir.dt.float32
BF16 = mybir.dt.bfloat16
I32 = mybir.dt.int32

_cache = {}


def _bcast(ap_obj, insert_at, count):
    """Return a bass.AP equal to ap_obj with a stride-0 dim inserted."""
    dims = list(ap_obj.ap)
    dims.insert(insert_at, [0, count])
    return bass.AP(tensor=ap_obj.tensor, offset=ap_obj.offset, ap=dims)


# ---------------- Launch A: argmax ----------------

def _build_argmax_nc(n_iters: int = 1):
    nc = bacc.Bacc("TRN2", target_bir_lowering=False, debug=False,
                   num_devices=NCORES)
    sol = nc.dram_tensor("sol", [NGRP, 128, NSUB * CPC], F32,
                         kind="ExternalInput")
    path_out = nc.dram_tensor("path_shard", [CPC], I32, kind="ExternalOutput")

    with tile.TileContext(nc) as tc:
        with ExitStack() as ctx:
            data_pool = ctx.enter_context(tc.tile_pool(name="data", bufs=2))
            midx_pool = ctx.enter_context(tc.tile_pool(name="midx", bufs=1))
            l_pool = ctx.enter_context(tc.tile_pool(name="l", bufs=1))
            par_pool = ctx.enter_context(tc.tile_pool(name="par", bufs=2))
            run_pool = ctx.enter_context(tc.tile_pool(name="run", bufs=2))
            const_pool = ctx.enter_context(tc.tile_pool(name="const", bufs=1))
            out_pool = ctx.enter_context(tc.tile_pool(name="out", bufs=1))

            # idx0[p, a] = p*8 + a + 1  (row+1 base within a group)
            idx0_i = const_pool.tile([128, NSUB], I32)
            nc.gpsimd.iota(idx0_i[:], pattern=[[1, NSUB]], base=1,
                           channel_multiplier=NSUB)
            idx0 = const_pool.tile([128, NSUB], F32)
            nc.vector.tensor_copy(idx0[:], idx0_i[:])

            loop_cm = (tc.For_i(0, n_iters, 1) if n_iters > 1
                       else contextlib.nullcontext())
            with loop_cm:
                B = R = None
                for g in range(NGRP):
                    # per-group row+1 constants: idx0 + g*1024
                    idxg = const_pool.tile([128, NSUB], F32, tag="idxg")
                    nc.vector.tensor_scalar(
                        out=idxg[:], in0=idx0[:], scalar1=float(g * 1024),
                        scalar2=None, op0=mybir.AluOpType.add)

                    T = data_pool.tile([128, NSUB, CPC], F32, tag="T")
                    nc.sync.dma_start(out=T[:], in_=sol[g].rearrange(
                        "p (a c) -> p a c", a=NSUB))

                    # value tree: 8 -> 4 -> 2 -> 1 (L1 then in-place)
                    L1 = l_pool.tile([128, 4, CPC], F32, tag="L1")
                    nc.vector.tensor_tensor(
                        out=L1[:], in0=T[:, 0:4, :], in1=T[:, 4:8, :],
                        op=mybir.AluOpType.max)
                    nc.vector.tensor_tensor(
                        out=L1[:, 0:2, :], in0=L1[:, 0:2, :],
                        in1=L1[:, 2:4, :], op=mybir.AluOpType.max)
                    nc.vector.tensor_tensor(
                        out=L1[:, 0, :], in0=L1[:, 0, :], in1=L1[:, 1, :],
                        op=mybir.AluOpType.max)

                    Bg = par_pool.tile([128, CPC], F32, tag="Bg")
                    nc.gpsimd.partition_all_reduce(
                        Bg[:], L1[:, 0, :], channels=128,
                        reduce_op=bass_isa.ReduceOp.max)

                    # masked row+1: midx = (T == Bg) * idxg, then fold max
                    midx = midx_pool.tile([128, NSUB, CPC], F32, tag="midx")
                    nc.vector.tensor_tensor(
                        out=midx[:], in0=T[:], in1=_bcast(Bg[:], 1, NSUB),
                        op=mybir.AluOpType.is_equal)
                    nc.vector.tensor_tensor(
                        out=midx[:], in0=midx[:], in1=_bcast(idxg[:], 2, CPC),
                        op=mybir.AluOpType.mult)
                    nc.vector.tensor_tensor(
                        out=midx[:, 0:4, :], in0=midx[:, 0:4, :],
                        in1=midx[:, 4:8, :], op=mybir.AluOpType.max)
                    nc.vector.tensor_tensor(
                        out=midx[:, 0:2, :], in0=midx[:, 0:2, :],
                        in1=midx[:, 2:4, :], op=mybir.AluOpType.max)
                    nc.vector.tensor_tensor(
                        out=midx[:, 0, :], in0=midx[:, 0, :],
                        in1=midx[:, 1, :], op=mybir.AluOpType.max)

                    Rg = par_pool.tile([128, CPC], F32, tag="Rg")
                    nc.gpsimd.partition_all_reduce(
                        Rg[:], midx[:, 0, :], channels=128,
                        reduce_op=bass_isa.ReduceOp.max)

                    # running combine on [1, CPC] slices
                    if g == 0:
                        B, R = Bg, Rg
                    else:
                        Bn = run_pool.tile([1, CPC], F32, tag="Bn")
                        nc.vector.tensor_tensor(
                            out=Bn[:], in0=B[0:1, :], in1=Bg[0:1, :],
                            op=mybir.AluOpType.max)
                        a1 = run_pool.tile([1, CPC], F32, tag="a1")
                        nc.vector.tensor_tensor(
                            out=a1[:], in0=B[0:1, :], in1=Bn[:],
                            op=mybir.AluOpType.is_equal)
                        nc.vector.tensor_tensor(
                            out=a1[:], in0=a1[:], in1=R[0:1, :],
                            op=mybir.AluOpType.mult)
                        a2 = run_pool.tile([1, CPC], F32, tag="a2")
                        nc.vector.tensor_tensor(
                            out=a2[:], in0=Bg[0:1, :], in1=Bn[:],
                            op=mybir.AluOpType.is_equal)
                        nc.vector.tensor_tensor(
                            out=a2[:], in0=a2[:], in1=Rg[0:1, :],
                            op=mybir.AluOpType.mult)
                        Rn = run_pool.tile([1, CPC], F32, tag="Rn")
                        nc.vector.tensor_tensor(
                            out=Rn[:], in0=a1[:], in1=a2[:],
                            op=mybir.AluOpType.max)
                        B, R = Bn, Rn

                # path = R - 1 -> int32
                pf = out_pool.tile([1, CPC], F32, tag="pf")
                nc.vector.tensor_scalar(
                    out=pf[:], in0=R[0:1, :] if R.shape[0] != 1 else R[:],
                    scalar1=-1.0, scalar2=None, op0=mybir.AluOpType.add)
                pi = out_pool.tile([1, CPC], I32, tag="pi")
                nc.vector.tensor_copy(pi[:], pf[:])
                nc.sync.dma_start(out=path_out[0:CPC], in_=pi[0:1, :])

    nc.compile()
    return nc


def _get_argmax_nc(n_iters: int = 1):
    key = ("argmax", n_iters)
    if key not in _cache:
        _cache[key] = _build_argmax_nc(n_iters)
    return _cache[key]


def run_argmax(solution_matrix: np.ndarray, n_iters: int = 1) -> np.ndarray:
    nc = _get_argmax_nc(n_iters)
    in_maps = []
    for k in range(NCORES):
        shard = np.ascontiguousarray(
            solution_matrix[:, k * CPC:(k + 1) * CPC])
        # [8192, 1024] -> [8, 128, 8*1024]; row = g*1024 + p*8 + a
        shard = shard.reshape(NGRP, 128, NSUB * CPC)
        in_maps.append({"sol": shard})
    res = run_bass_kernel_spmd(nc, in_maps, core_ids=list(range(NCORES)))
    path = np.concatenate([res.results[k]["path_shard"]
                           for k in range(NCORES)])
    return path.astype(np.int32)


# ---------------- Launch B: gather + sum ----------------

GROWS = N // NCORES * N // 64  # 131072 blocks per core's row shard


def _build_gather_nc(n_iters: int = 1):
    """SPMD: core k holds cost rows [1024k, 1024(k+1)) viewed [131072, 64]
    plus the full path (8193 with wrap) and rbase = k*2^23.  Every core
    attempts all 8192 terms; block indices outside its row shard land out
    of bounds and are silently skipped (dest pre-zeroed), so each term is
    summed by exactly one core.  Output part [1] f32; host adds the 8."""
    G = N // 128  # 64 terms per partition
    nc = bacc.Bacc("TRN2", target_bir_lowering=False, debug=False,
                   num_devices=NCORES)
    cost = nc.dram_tensor("cost", [GROWS, 64], F32, kind="ExternalInput")
    pseg = nc.dram_tensor("pseg", [N + 1], I32, kind="ExternalInput")
    rbase = nc.dram_tensor("rbase", [128, 1], I32, kind="ExternalInput")
    out = nc.dram_tensor("part", [1], F32, kind="ExternalOutput")

    with tile.TileContext(nc) as tc:
        with ExitStack() as ctx:
            pool = ctx.enter_context(tc.tile_pool(name="p", bufs=2))
            const_pool = ctx.enter_context(tc.tile_pool(name="c", bufs=1))

            # iota64[p, c] = c  (same in every partition)
            io64_i = const_pool.tile([128, 64], I32)
            nc.gpsimd.iota(io64_i[:], pattern=[[1, 64]], base=0,
                           channel_multiplier=0)
            io64 = const_pool.tile([128, 64], F32)
            nc.vector.tensor_copy(io64[:], io64_i[:])
            rb = const_pool.tile([128, 1], I32)
            nc.sync.dma_start(out=rb[:], in_=rbase[:, :])

            loop_cm = (tc.For_i(0, n_iters, 1) if n_iters > 1
                       else contextlib.nullcontext())
            with loop_cm:
                # ptx[p, j] = pseg[p*G + j], j in [0, G]  (overlapping rows)
                ptx = pool.tile([128, G + 1], I32, tag="ptx")
                full = pseg[:]
                src = bass.AP(tensor=full.tensor, offset=full.offset,
                              ap=[[G, 128], [1, G + 1]])
                nc.sync.dma_start(out=ptx[:], in_=src)

                # flat = (pt << 13) | nx ; rem = flat & 63
                # blk = (flat - rbase) >> 6 (logical: negatives go huge ->
                # out of bounds under signed or unsigned compare)
                fl = pool.tile([128, G], I32, tag="fl")
                nc.vector.tensor_scalar(
                    out=fl[:], in0=ptx[:, 0:G], scalar1=13, scalar2=None,
                    op0=mybir.AluOpType.logical_shift_left)
                nc.vector.tensor_tensor(
                    out=fl[:], in0=fl[:], in1=ptx[:, 1:G + 1],
                    op=mybir.AluOpType.bitwise_or)
                remi = pool.tile([128, G], I32, tag="remi")
                nc.vector.tensor_scalar(
                    out=remi[:], in0=fl[:], scalar1=63, scalar2=None,
                    op0=mybir.AluOpType.bitwise_and)
                rem = pool.tile([128, G], F32, tag="rem")
                nc.vector.tensor_copy(rem[:], remi[:])
                blk = pool.tile([128, G], I32, tag="blk")
                rb_b = bass.AP(tensor=rb[:].tensor, offset=rb[:].offset,
                               ap=[rb[:].ap[0], [0, G]])
                nc.vector.tensor_tensor(
                    out=blk[:], in0=fl[:], in1=rb_b,
                    op=mybir.AluOpType.subtract)
                nc.vector.tensor_scalar(
                    out=blk[:], in0=blk[:], scalar1=6, scalar2=None,
                    op0=mybir.AluOpType.logical_shift_right)

                vals = pool.tile([128, G, 64], F32, tag="vals")
                nc.vector.memset(vals[:], 0.0)
                for g in range(G):
                    nc.gpsimd.indirect_dma_start(
                        out=vals[:, g, :], out_offset=None,
                        in_=cost[:, :],
                        in_offset=bass.IndirectOffsetOnAxis(
                            ap=blk[:, g:g + 1], axis=0),
                        bounds_check=GROWS - 1,
                        oob_is_err=False)

                # mask[p,g,c] = (io64[p,c] == rem[p,g]) ; dot with vals
                mask = pool.tile([128, G, 64], F32, tag="mask")
                nc.vector.tensor_tensor(
                    out=mask[:], in0=_bcast(io64[:], 1, G),
                    in1=_bcast(rem[:], 2, 64), op=mybir.AluOpType.is_equal)
                nc.vector.tensor_tensor(
                    out=mask[:], in0=mask[:], in1=vals[:],
                    op=mybir.AluOpType.mult)
                s1 = pool.tile([128, 1], F32, tag="s1")
                nc.vector.reduce_sum(
                    s1[:], mask[:].rearrange("p g c -> p (g c)"),
                    axis=mybir.AxisListType.X)
                s2 = pool.tile([128, 1], F32, tag="s2")
                nc.gpsimd.partition_all_reduce(
                    s2[:], s1[:], channels=128,
                    reduce_op=bass_isa.ReduceOp.add)
                nc.sync.dma_start(out=out[0:1], in_=s2[0:1, 0:1])

    nc.compile()
    return nc


def _get_gather_nc(n_iters: int = 1):
    key = ("gather", n_iters)
    if key not in _cache:
        _cache[key] = _build_gather_nc(n_iters)
    return _cache[key]


def run_gather(cost_matrix: np.ndarray, path: np.ndarray,
               n_iters: int = 1) -> np.ndarray:
    nc = _get_gather_nc(n_iters)
    cost_c = np.ascontiguousarray(cost_matrix)
    pfull = np.concatenate([path.astype(np.int32), path[:1].astype(np.int32)])
    in_maps = []
    for k in range(NCORES):
        shard = cost_c[k * (N // NCORES):(k + 1) * (N // NCORES), :]
        in_maps.append({
            "cost": shard.reshape(GROWS, 64),
            "pseg": pfull,
            "rbase": np.full((128, 1), k * (N // NCORES) * N,
                             dtype=np.int32),
        })
    res = run_bass_kernel_spmd(nc, in_maps, core_ids=list(range(NCORES)))
    total = np.float32(0.0)
    for k in range(NCORES):
        total += np.asarray(res.results[k]["part"], dtype=np.float32)[0]
    return np.asarray([total], dtype=np.float32)


def kernel(solution_matrix: np.ndarray, cost_matrix: np.ndarray) -> np.ndarray:
    path = run_argmax(solution_matrix)
    return run_gather(cost_matrix, path)


if __name__ == "__main__":
    rng = np.random.default_rng(0)
    sol = rng.standard_normal((N, N), dtype=np.float32)
    cm = rng.random((N, N), dtype=np.float32)
    path = run_argmax(sol)
    want = sol.argmax(axis=0)
    print("argmax match:", np.array_equal(path, want),
          int((path != want).sum()), "mismatches")
    got = run_gather(cm, path)
    nxt = np.roll(want, -1)
    exp = cm[want, nxt].sum()
    print("cost:", got, "expected:", exp,
          "rel:", abs(got[0] - exp) / abs(exp))


# revision 7
# speedup vs baseline: 24.6617x; 1.2256x over previous
"""Trainium2 Bass kernel v2 for nn_Custom_Loss_84937273246180.

reference:
    path = argmax(solution_matrix, axis=0)        # [8192] int
    nxt  = roll(path, -1)
    out  = sum(cost_matrix[path, nxt])            # [1] f32

Strategy (8 NeuronCores, two launches):

Launch A (argmax, column-sharded): core k owns columns [1024k, 1024(k+1)).
  Host reshapes its [8192, 1024] shard to [8 groups, 128 p, 8192] where
  row = g*1024 + p*8 + a (pure row-major reshape; per-(g,p) DRAM run is a
  contiguous 32KB block -> 128 big descriptors per DMA).
  Per group: one 4MB DMA, DVE max-tree to the group column max, gpsimd
  partition_all_reduce to replicate it, is_equal mask * (row+1) and a max
  fold + partition_all_reduce to extract the argmax row, then a running
  cross-group combine on [1, 1024] slices.  No PE, no PSUM, ~15 instrs
  per group, folds done in place to fit SBUF.

Launch B (gather, term-sharded): every core gets the full cost matrix
  (viewed [1048576, 64]) plus a 1025-entry path segment (1024 terms + the
  wrap element, host-sliced).  Each core gathers 256B blocks containing
  its 1024 terms via 8 indirect DMAs (offset AP [128, 1], D=64 -- the
  fast, hardware-verified form), selects the in-block element with an
  iota/is_equal mask, reduces, and emits a [1] partial; host adds the 8
  partials.
"""

import contextlib
import numpy as np
from contextlib import ExitStack

import concourse.bass as bass
import concourse.bacc as bacc
import concourse.tile as tile
from concourse import mybir
from concourse import bass_isa
from concourse.bass_utils import run_bass_kernel_spmd

N = 8192
NCORES = 8
CPC = N // NCORES        # columns per core = 1024
NGRP = 8                 # row groups per core
NSUB = 8                 # sub-rows per partition per group
# row = g*1024 + p*8 + a

F32 = mybir.dt.float32
BF16 = mybir.dt.bfloat16
I32 = mybir.dt.int32

_cache = {}


def _bcast(ap_obj, insert_at, count):
    """Return a bass.AP equal to ap_obj with a stride-0 dim inserted."""
    dims = list(ap_obj.ap)
    dims.insert(insert_at, [0, count])
    return bass.AP(tensor=ap_obj.tensor, offset=ap_obj.offset, ap=dims)


# ---------------- Launch A: argmax ----------------

def _build_argmax_nc(n_iters: int = 1):
    nc = bacc.Bacc("TRN2", target_bir_lowering=False, debug=False,
                   num_devices=NCORES)
    sol = nc.dram_tensor("sol", [NGRP, 128, NSUB * CPC], F32,
                         kind="ExternalInput")
    path_out = nc.dram_tensor("path_shard", [CPC], I32, kind="ExternalOutput")

    with tile.TileContext(nc) as tc:
        with ExitStack() as ctx:
            data_pool = ctx.enter_context(tc.tile_pool(name="data", bufs=2))
            midx_pool = ctx.enter_context(tc.tile_pool(name="midx", bufs=1))
            l_pool = ctx.enter_context(tc.tile_pool(name="l", bufs=1))
            par_pool = ctx.enter_context(tc.tile_pool(name="par", bufs=2))
            run_pool = ctx.enter_context(tc.tile_pool(name="run", bufs=2))
            const_pool = ctx.enter_context(tc.tile_pool(name="const", bufs=1))
            out_pool = ctx.enter_context(tc.tile_pool(name="out", bufs=1))

            # idx0[p, a] = p*8 + a + 1  (row+1 base within a group)
            idx0_i = const_pool.tile([128, NSUB], I32)
            nc.gpsimd.iota(idx0_i[:], pattern=[[1, NSUB]], base=1,
                           channel_multiplier=NSUB)
            idx0 = const_pool.tile([128, NSUB], F32)
            nc.vector.tensor_copy(idx0[:], idx0_i[:])

            loop_cm = (tc.For_i(0, n_iters, 1) if n_iters > 1
                       else contextlib.nullcontext())
            with loop_cm:
                B = R = None
                for g in range(NGRP):
                    # per-group row+1 constants: idx0 + g*1024
                    idxg = const_pool.tile([128, NSUB], F32, tag="idxg")
                    nc.vector.tensor_scalar(
                        out=idxg[:], in0=idx0[:], scalar1=float(g * 1024),
                        scalar2=None, op0=mybir.AluOpType.add)

                    T = data_pool.tile([128, NSUB, CPC], F32, tag="T")
                    nc.sync.dma_start(out=T[:], in_=sol[g].rearrange(
                        "p (a c) -> p a c", a=NSUB))

                    # value tree: 8 -> 4 -> 2 -> 1 (L1 then in-place)
                    L1 = l_pool.tile([128, 4, CPC], F32, tag="L1")
                    nc.vector.tensor_tensor(
                        out=L1[:], in0=T[:, 0:4, :], in1=T[:, 4:8, :],
                        op=mybir.AluOpType.max)
                    nc.vector.tensor_tensor(
                        out=L1[:, 0:2, :], in0=L1[:, 0:2, :],
                        in1=L1[:, 2:4, :], op=mybir.AluOpType.max)
                    nc.vector.tensor_tensor(
                        out=L1[:, 0, :], in0=L1[:, 0, :], in1=L1[:, 1, :],
                        op=mybir.AluOpType.max)

                    Bg = par_pool.tile([128, CPC], F32, tag="Bg")
                    nc.gpsimd.partition_all_reduce(
                        Bg[:], L1[:, 0, :], channels=128,
                        reduce_op=bass_isa.ReduceOp.max)

                    # masked row+1: midx = (T == Bg) * idxg, then fold max
                    midx = midx_pool.tile([128, NSUB, CPC], F32, tag="midx")
                    nc.vector.tensor_tensor(
                        out=midx[:], in0=T[:], in1=_bcast(Bg[:], 1, NSUB),
                        op=mybir.AluOpType.is_equal)
                    nc.vector.tensor_tensor(
                        out=midx[:], in0=midx[:], in1=_bcast(idxg[:], 2, CPC),
                        op=mybir.AluOpType.mult)
                    nc.vector.tensor_tensor(
                        out=midx[:, 0:4, :], in0=midx[:, 0:4, :],
                        in1=midx[:, 4:8, :], op=mybir.AluOpType.max)
                    nc.vector.tensor_tensor(
                        out=midx[:, 0:2, :], in0=midx[:, 0:2, :],
                        in1=midx[:, 2:4, :], op=mybir.AluOpType.max)
                    nc.vector.tensor_tensor(
                        out=midx[:, 0, :], in0=midx[:, 0, :],
                        in1=midx[:, 1, :], op=mybir.AluOpType.max)

                    Rg = par_pool.tile([128, CPC], F32, tag="Rg")
                    nc.gpsimd.partition_all_reduce(
                        Rg[:], midx[:, 0, :], channels=128,
                        reduce_op=bass_isa.ReduceOp.max)

                    # running combine on [1, CPC] slices
                    if g == 0:
                        B, R = Bg, Rg
                    else:
                        Bn = run_pool.tile([1, CPC], F32, tag="Bn")
                        nc.vector.tensor_tensor(
                            out=Bn[:], in0=B[0:1, :], in1=Bg[0:1, :],
                            op=mybir.AluOpType.max)
                        a1 = run_pool.tile([1, CPC], F32, tag="a1")
                        nc.vector.tensor_tensor(
                            out=a1[:], in0=B[0:1, :], in1=Bn[:],
                            op=mybir.AluOpType.is_equal)
                        nc.vector.tensor_tensor(
                            out=a1[:], in0=a1[:], in1=R[0:1, :],
                            op=mybir.AluOpType.mult)
                        a2 = run_pool.tile([1, CPC], F32, tag="a2")
                        nc.vector.tensor_tensor(
                            out=a2[:], in0=Bg[0:1, :], in1=Bn[:],
                            op=mybir.AluOpType.is_equal)
                        nc.vector.tensor_tensor(
                            out=a2[:], in0=a2[:], in1=Rg[0:1, :],
                            op=mybir.AluOpType.mult)
                        Rn = run_pool.tile([1, CPC], F32, tag="Rn")
                        nc.vector.tensor_tensor(
                            out=Rn[:], in0=a1[:], in1=a2[:],
                            op=mybir.AluOpType.max)
                        B, R = Bn, Rn

                # path = R - 1 -> int32
                pf = out_pool.tile([1, CPC], F32, tag="pf")
                nc.vector.tensor_scalar(
                    out=pf[:], in0=R[0:1, :] if R.shape[0] != 1 else R[:],
                    scalar1=-1.0, scalar2=None, op0=mybir.AluOpType.add)
                pi = out_pool.tile([1, CPC], I32, tag="pi")
                nc.vector.tensor_copy(pi[:], pf[:])
                nc.sync.dma_start(out=path_out[0:CPC], in_=pi[0:1, :])

    nc.compile()
    return nc


def _get_argmax_nc(n_iters: int = 1):
    key = ("argmax", n_iters)
    if key not in _cache:
        _cache[key] = _build_argmax_nc(n_iters)
    return _cache[key]


def run_argmax(solution_matrix: np.ndarray, n_iters: int = 1) -> np.ndarray:
    nc = _get_argmax_nc(n_iters)
    in_maps = []
    for k in range(NCORES):
        shard = np.ascontiguousarray(
            solution_matrix[:, k * CPC:(k + 1) * CPC])
        # [8192, 1024] -> [8, 128, 8*1024]; row = g*1024 + p*8 + a
        shard = shard.reshape(NGRP, 128, NSUB * CPC)
        in_maps.append({"sol": shard})
    res = run_bass_kernel_spmd(nc, in_maps, core_ids=list(range(NCORES)))
    path = np.concatenate([res.results[k]["path_shard"]
                           for k in range(NCORES)])
    return path.astype(np.int32)


# ---------------- Launch B: gather + sum ----------------

GROWS = N // NCORES * N // 64  # 131072 blocks per core's row shard


def _build_gather_nc(n_iters: int = 1):
    """SPMD: core k holds cost rows [1024k, 1024(k+1)) viewed [131072, 64]
    plus the full path (8193 with wrap) and rbase = k*2^23.  Every core
    attempts all 8192 terms; block indices outside its row shard land out
    of bounds and are silently skipped (dest pre-zeroed), so each term is
    summed by exactly one core.  Output part [1] f32; host adds the 8."""
    G = N // 128  # 64 terms per partition
    nc = bacc.Bacc("TRN2", target_bir_lowering=False, debug=False,
                   num_devices=NCORES)
    cost = nc.dram_tensor("cost", [GROWS, 64], F32, kind="ExternalInput")
    pseg = nc.dram_tensor("pseg", [N + 1], I32, kind="ExternalInput")
    rbase = nc.dram_tensor("rbase", [128, 1], I32, kind="ExternalInput")
    out = nc.dram_tensor("part", [1], F32, kind="ExternalOutput")

    with tile.TileContext(nc) as tc:
        with ExitStack() as ctx:
            pool = ctx.enter_context(tc.tile_pool(name="p", bufs=2))
            const_pool = ctx.enter_context(tc.tile_pool(name="c", bufs=1))

            # iota64[p, c] = c  (same in every partition)
            io64_i = const_pool.tile([128, 64], I32)
            nc.gpsimd.iota(io64_i[:], pattern=[[1, 64]], base=0,
                           channel_multiplier=0)
            io64 = const_pool.tile([128, 64], F32)
            nc.vector.tensor_copy(io64[:], io64_i[:])
            rb = const_pool.tile([128, 1], I32)
            nc.sync.dma_start(out=rb[:], in_=rbase[:, :])

            loop_cm = (tc.For_i(0, n_iters, 1) if n_iters > 1
                       else contextlib.nullcontext())
            with loop_cm:
                # ptx[p, j] = pseg[p*G + j], j in [0, G]  (overlapping rows)
                ptx = pool.tile([128, G + 1], I32, tag="ptx")
                full = pseg[:]
                src = bass.AP(tensor=full.tensor, offset=full.offset,
                              ap=[[G, 128], [1, G + 1]])
                nc.sync.dma_start(out=ptx[:], in_=src)

                # global block = (flat >> 6) = pt*128 + (nx >> 6); local
                # block = global - k*2^17.  All intermediates stay < 2^21
                # so an f32-datapath int ALU cannot round them (int32
                # tensor_tensor beyond 2^24 was observed to round).
                b1 = pool.tile([128, G], I32, tag="b1")
                nc.vector.tensor_scalar(
                    out=b1[:], in0=ptx[:, 0:G], scalar1=7, scalar2=None,
                    op0=mybir.AluOpType.logical_shift_left)
                b2 = pool.tile([128, G], I32, tag="b2")
                nc.vector.tensor_scalar(
                    out=b2[:], in0=ptx[:, 1:G + 1], scalar1=6, scalar2=None,
                    op0=mybir.AluOpType.logical_shift_right)
                blk = pool.tile([128, G], I32, tag="blk")
                nc.vector.tensor_tensor(
                    out=blk[:], in0=b1[:], in1=b2[:],
                    op=mybir.AluOpType.add)
                rb_b = bass.AP(tensor=rb[:].tensor, offset=rb[:].offset,
                               ap=[rb[:].ap[0], [0, G]])
                nc.vector.tensor_tensor(
                    out=blk[:], in0=blk[:], in1=rb_b,
                    op=mybir.AluOpType.subtract)
                # rem = nx & 63 (low 6 bits of flat come from nx)
                remi = pool.tile([128, G], I32, tag="remi")
                nc.vector.tensor_scalar(
                    out=remi[:], in0=ptx[:, 1:G + 1], scalar1=63,
                    scalar2=None, op0=mybir.AluOpType.bitwise_and)
                rem = pool.tile([128, G], F32, tag="rem")
                nc.vector.tensor_copy(rem[:], remi[:])

                vals = pool.tile([128, G, 64], F32, tag="vals")
                nc.vector.memset(vals[:], 0.0)
                for g in range(G):
                    nc.gpsimd.indirect_dma_start(
                        out=vals[:, g, :], out_offset=None,
                        in_=cost[:, :],
                        in_offset=bass.IndirectOffsetOnAxis(
                            ap=blk[:, g:g + 1], axis=0),
                        bounds_check=GROWS - 1,
                        oob_is_err=False)

                # mask[p,g,c] = (io64[p,c] == rem[p,g]) ; dot with vals
                mask = pool.tile([128, G, 64], F32, tag="mask")
                nc.vector.tensor_tensor(
                    out=mask[:], in0=_bcast(io64[:], 1, G),
                    in1=_bcast(rem[:], 2, 64), op=mybir.AluOpType.is_equal)
                nc.vector.tensor_tensor(
                    out=mask[:], in0=mask[:], in1=vals[:],
                    op=mybir.AluOpType.mult)
                s1 = pool.tile([128, 1], F32, tag="s1")
                nc.vector.reduce_sum(
                    s1[:], mask[:].rearrange("p g c -> p (g c)"),
                    axis=mybir.AxisListType.X)
                s2 = pool.tile([128, 1], F32, tag="s2")
                nc.gpsimd.partition_all_reduce(
                    s2[:], s1[:], channels=128,
                    reduce_op=bass_isa.ReduceOp.add)
                nc.sync.dma_start(out=out[0:1], in_=s2[0:1, 0:1])

    nc.compile()
    return nc


def _get_gather_nc(n_iters: int = 1):
    key = ("gather", n_iters)
    if key not in _cache:
        _cache[key] = _build_gather_nc(n_iters)
    return _cache[key]


def run_gather(cost_matrix: np.ndarray, path: np.ndarray,
               n_iters: int = 1) -> np.ndarray:
    nc = _get_gather_nc(n_iters)
    cost_c = np.ascontiguousarray(cost_matrix)
    pfull = np.concatenate([path.astype(np.int32), path[:1].astype(np.int32)])
    in_maps = []
    for k in range(NCORES):
        shard = cost_c[k * (N // NCORES):(k + 1) * (N // NCORES), :]
        in_maps.append({
            "cost": shard.reshape(GROWS, 64),
            "pseg": pfull,
            "rbase": np.full((128, 1), k * GROWS, dtype=np.int32),
        })
    res = run_bass_kernel_spmd(nc, in_maps, core_ids=list(range(NCORES)))
    total = np.float32(0.0)
    for k in range(NCORES):
        total += np.asarray(res.results[k]["part"], dtype=np.float32)[0]
    return np.asarray([total], dtype=np.float32)


def kernel(solution_matrix: np.ndarray, cost_matrix: np.ndarray) -> np.ndarray:
    path = run_argmax(solution_matrix)
    return run_gather(cost_matrix, path)


if __name__ == "__main__":
    rng = np.random.default_rng(0)
    sol = rng.standard_normal((N, N), dtype=np.float32)
    cm = rng.random((N, N), dtype=np.float32)
    path = run_argmax(sol)
    want = sol.argmax(axis=0)
    print("argmax match:", np.array_equal(path, want),
          int((path != want).sum()), "mismatches")
    got = run_gather(cm, path)
    nxt = np.roll(want, -1)
    exp = cm[want, nxt].sum()
    print("cost:", got, "expected:", exp,
          "rel:", abs(got[0] - exp) / abs(exp))


# revision 8
# speedup vs baseline: 34.9869x; 1.4187x over previous
"""Trainium2 Bass kernel v2 for nn_Custom_Loss_84937273246180.

reference:
    path = argmax(solution_matrix, axis=0)        # [8192] int
    nxt  = roll(path, -1)
    out  = sum(cost_matrix[path, nxt])            # [1] f32

Strategy (8 NeuronCores, two launches):

Launch A (argmax, column-sharded): core k owns columns [1024k, 1024(k+1)).
  Host reshapes its [8192, 1024] shard to [8 groups, 128 p, 8192] where
  row = g*1024 + p*8 + a (pure row-major reshape; per-(g,p) DRAM run is a
  contiguous 32KB block -> 128 big descriptors per DMA).
  Per group: one 4MB DMA, DVE max-tree to the group column max, gpsimd
  partition_all_reduce to replicate it, is_equal mask * (row+1) and a max
  fold + partition_all_reduce to extract the argmax row, then a running
  cross-group combine on [1, 1024] slices.  No PE, no PSUM, ~15 instrs
  per group, folds done in place to fit SBUF.

Launch B (gather, term-sharded): every core gets the full cost matrix
  (viewed [1048576, 64]) plus a 1025-entry path segment (1024 terms + the
  wrap element, host-sliced).  Each core gathers 256B blocks containing
  its 1024 terms via 8 indirect DMAs (offset AP [128, 1], D=64 -- the
  fast, hardware-verified form), selects the in-block element with an
  iota/is_equal mask, reduces, and emits a [1] partial; host adds the 8
  partials.
"""

import contextlib
import numpy as np
from contextlib import ExitStack

import concourse.bass as bass
import concourse.bacc as bacc
import concourse.tile as tile
from concourse import mybir
from concourse import bass_isa
from concourse.bass_utils import run_bass_kernel_spmd

N = 8192
NCORES = 8
CPC = N // NCORES        # columns per core = 1024
NGRP = 8                 # row groups per core
NSUB = 8                 # sub-rows per partition per group
# row = g*1024 + p*8 + a

F32 = mybir.dt.float32
BF16 = mybir.dt.bfloat16
I32 = mybir.dt.int32

_cache = {}


def _bcast(ap_obj, insert_at, count):
    """Return a bass.AP equal to ap_obj with a stride-0 dim inserted."""
    dims = list(ap_obj.ap)
    dims.insert(insert_at, [0, count])
    return bass.AP(tensor=ap_obj.tensor, offset=ap_obj.offset, ap=dims)


# ---------------- Launch A: argmax ----------------

def _build_argmax_nc(n_iters: int = 1):
    nc = bacc.Bacc("TRN2", target_bir_lowering=False, debug=False,
                   num_devices=NCORES)
    sol = nc.dram_tensor("sol", [NGRP, 128, NSUB * CPC], F32,
                         kind="ExternalInput")
    path_out = nc.dram_tensor("path_shard", [CPC], I32, kind="ExternalOutput")

    with tile.TileContext(nc) as tc:
        with ExitStack() as ctx:
            data_pool = ctx.enter_context(tc.tile_pool(name="data", bufs=2))
            midx_pool = ctx.enter_context(tc.tile_pool(name="midx", bufs=1))
            l_pool = ctx.enter_context(tc.tile_pool(name="l", bufs=1))
            par_pool = ctx.enter_context(tc.tile_pool(name="par", bufs=2))
            run_pool = ctx.enter_context(tc.tile_pool(name="run", bufs=2))
            const_pool = ctx.enter_context(tc.tile_pool(name="const", bufs=1))
            out_pool = ctx.enter_context(tc.tile_pool(name="out", bufs=1))

            # idx0[p, a] = p*8 + a + 1  (row+1 base within a group)
            idx0_i = const_pool.tile([128, NSUB], I32)
            nc.gpsimd.iota(idx0_i[:], pattern=[[1, NSUB]], base=1,
                           channel_multiplier=NSUB)
            idx0 = const_pool.tile([128, NSUB], F32)
            nc.vector.tensor_copy(idx0[:], idx0_i[:])

            loop_cm = (tc.For_i(0, n_iters, 1) if n_iters > 1
                       else contextlib.nullcontext())
            with loop_cm:
                B = R = None
                for g in range(NGRP):
                    # per-group row+1 constants: idx0 + g*1024
                    idxg = const_pool.tile([128, NSUB], F32, tag="idxg")
                    nc.vector.tensor_scalar(
                        out=idxg[:], in0=idx0[:], scalar1=float(g * 1024),
                        scalar2=None, op0=mybir.AluOpType.add)

                    T = data_pool.tile([128, NSUB, CPC], F32, tag="T")
                    nc.sync.dma_start(out=T[:], in_=sol[g].rearrange(
                        "p (a c) -> p a c", a=NSUB))

                    # value tree: 8 -> 4 -> 2 -> 1 (L1 then in-place)
                    L1 = l_pool.tile([128, 4, CPC], F32, tag="L1")
                    nc.vector.tensor_tensor(
                        out=L1[:], in0=T[:, 0:4, :], in1=T[:, 4:8, :],
                        op=mybir.AluOpType.max)
                    nc.vector.tensor_tensor(
                        out=L1[:, 0:2, :], in0=L1[:, 0:2, :],
                        in1=L1[:, 2:4, :], op=mybir.AluOpType.max)
                    nc.vector.tensor_tensor(
                        out=L1[:, 0, :], in0=L1[:, 0, :], in1=L1[:, 1, :],
                        op=mybir.AluOpType.max)

                    Bg = par_pool.tile([128, CPC], F32, tag="Bg")
                    nc.gpsimd.partition_all_reduce(
                        Bg[:], L1[:, 0, :], channels=128,
                        reduce_op=bass_isa.ReduceOp.max)

                    # masked row+1: midx = (T == Bg) * idxg, then fold max
                    midx = midx_pool.tile([128, NSUB, CPC], F32, tag="midx")
                    nc.vector.tensor_tensor(
                        out=midx[:], in0=T[:], in1=_bcast(Bg[:], 1, NSUB),
                        op=mybir.AluOpType.is_equal)
                    nc.vector.tensor_tensor(
                        out=midx[:], in0=midx[:], in1=_bcast(idxg[:], 2, CPC),
                        op=mybir.AluOpType.mult)
                    nc.vector.tensor_tensor(
                        out=midx[:, 0:4, :], in0=midx[:, 0:4, :],
                        in1=midx[:, 4:8, :], op=mybir.AluOpType.max)
                    nc.vector.tensor_tensor(
                        out=midx[:, 0:2, :], in0=midx[:, 0:2, :],
                        in1=midx[:, 2:4, :], op=mybir.AluOpType.max)
                    nc.vector.tensor_tensor(
                        out=midx[:, 0, :], in0=midx[:, 0, :],
                        in1=midx[:, 1, :], op=mybir.AluOpType.max)

                    Rg = par_pool.tile([128, CPC], F32, tag="Rg")
                    nc.gpsimd.partition_all_reduce(
                        Rg[:], midx[:, 0, :], channels=128,
                        reduce_op=bass_isa.ReduceOp.max)

                    # running combine on [1, CPC] slices
                    if g == 0:
                        B, R = Bg, Rg
                    else:
                        Bn = run_pool.tile([1, CPC], F32, tag="Bn")
                        nc.vector.tensor_tensor(
                            out=Bn[:], in0=B[0:1, :], in1=Bg[0:1, :],
                            op=mybir.AluOpType.max)
                        a1 = run_pool.tile([1, CPC], F32, tag="a1")
                        nc.vector.tensor_tensor(
                            out=a1[:], in0=B[0:1, :], in1=Bn[:],
                            op=mybir.AluOpType.is_equal)
                        nc.vector.tensor_tensor(
                            out=a1[:], in0=a1[:], in1=R[0:1, :],
                            op=mybir.AluOpType.mult)
                        a2 = run_pool.tile([1, CPC], F32, tag="a2")
                        nc.vector.tensor_tensor(
                            out=a2[:], in0=Bg[0:1, :], in1=Bn[:],
                            op=mybir.AluOpType.is_equal)
                        nc.vector.tensor_tensor(
                            out=a2[:], in0=a2[:], in1=Rg[0:1, :],
                            op=mybir.AluOpType.mult)
                        Rn = run_pool.tile([1, CPC], F32, tag="Rn")
                        nc.vector.tensor_tensor(
                            out=Rn[:], in0=a1[:], in1=a2[:],
                            op=mybir.AluOpType.max)
                        B, R = Bn, Rn

                # path = R - 1 -> int32
                pf = out_pool.tile([1, CPC], F32, tag="pf")
                nc.vector.tensor_scalar(
                    out=pf[:], in0=R[0:1, :] if R.shape[0] != 1 else R[:],
                    scalar1=-1.0, scalar2=None, op0=mybir.AluOpType.add)
                pi = out_pool.tile([1, CPC], I32, tag="pi")
                nc.vector.tensor_copy(pi[:], pf[:])
                nc.sync.dma_start(out=path_out[0:CPC], in_=pi[0:1, :])

    nc.compile()
    return nc


def _get_argmax_nc(n_iters: int = 1):
    key = ("argmax", n_iters)
    if key not in _cache:
        _cache[key] = _build_argmax_nc(n_iters)
    return _cache[key]


def run_argmax(solution_matrix: np.ndarray, n_iters: int = 1) -> np.ndarray:
    nc = _get_argmax_nc(n_iters)
    in_maps = []
    for k in range(NCORES):
        shard = np.ascontiguousarray(
            solution_matrix[:, k * CPC:(k + 1) * CPC])
        # [8192, 1024] -> [8, 128, 8*1024]; row = g*1024 + p*8 + a
        shard = shard.reshape(NGRP, 128, NSUB * CPC)
        in_maps.append({"sol": shard})
    res = run_bass_kernel_spmd(nc, in_maps, core_ids=list(range(NCORES)))
    path = np.concatenate([res.results[k]["path_shard"]
                           for k in range(NCORES)])
    return path.astype(np.int32)


# ---------------- Launch B: gather + sum ----------------

GROWS = N // NCORES * N // 64  # 131072 blocks per core's row shard


def _build_gather_nc(n_iters: int = 1):
    """SPMD: core k holds cost rows [1024k, 1024(k+1)) viewed [131072, 64]
    plus the full path (8193 with wrap) and rbase = k*2^23.  Every core
    attempts all 8192 terms; block indices outside its row shard land out
    of bounds and are silently skipped (dest pre-zeroed), so each term is
    summed by exactly one core.  Output part [1] f32; host adds the 8."""
    G = N // 128  # 64 terms per partition
    nc = bacc.Bacc("TRN2", target_bir_lowering=False, debug=False,
                   num_devices=NCORES)
    cost = nc.dram_tensor("cost", [GROWS, 64], F32, kind="ExternalInput")
    pseg = nc.dram_tensor("pseg", [N + 1], I32, kind="ExternalInput")
    rbase = nc.dram_tensor("rbase", [128, 1], I32, kind="ExternalInput")
    out = nc.dram_tensor("part", [1], F32, kind="ExternalOutput")

    with tile.TileContext(nc) as tc:
        with ExitStack() as ctx:
            pool = ctx.enter_context(tc.tile_pool(name="p", bufs=2))
            const_pool = ctx.enter_context(tc.tile_pool(name="c", bufs=1))

            # iota64[p, c] = c  (same in every partition)
            io64_i = const_pool.tile([128, 64], I32)
            nc.gpsimd.iota(io64_i[:], pattern=[[1, 64]], base=0,
                           channel_multiplier=0)
            io64 = const_pool.tile([128, 64], F32)
            nc.vector.tensor_copy(io64[:], io64_i[:])
            rb = const_pool.tile([128, 1], I32)
            nc.sync.dma_start(out=rb[:], in_=rbase[:, :])

            loop_cm = (tc.For_i(0, n_iters, 1) if n_iters > 1
                       else contextlib.nullcontext())
            with loop_cm:
                # ptx[p, j] = pseg[p*G + j], j in [0, G]  (overlapping rows)
                ptx = pool.tile([128, G + 1], I32, tag="ptx")
                full = pseg[:]
                src = bass.AP(tensor=full.tensor, offset=full.offset,
                              ap=[[G, 128], [1, G + 1]])
                nc.sync.dma_start(out=ptx[:], in_=src)

                # global block = (flat >> 6) = pt*128 + (nx >> 6); local
                # block = global - k*2^17.  All intermediates stay < 2^21
                # so an f32-datapath int ALU cannot round them (int32
                # tensor_tensor beyond 2^24 was observed to round).
                b1 = pool.tile([128, G], I32, tag="b1")
                nc.vector.tensor_scalar(
                    out=b1[:], in0=ptx[:, 0:G], scalar1=7, scalar2=None,
                    op0=mybir.AluOpType.logical_shift_left)
                b2 = pool.tile([128, G], I32, tag="b2")
                nc.vector.tensor_scalar(
                    out=b2[:], in0=ptx[:, 1:G + 1], scalar1=6, scalar2=None,
                    op0=mybir.AluOpType.logical_shift_right)
                blk = pool.tile([128, G], I32, tag="blk")
                nc.vector.tensor_tensor(
                    out=blk[:], in0=b1[:], in1=b2[:],
                    op=mybir.AluOpType.add)
                rb_b = bass.AP(tensor=rb[:].tensor, offset=rb[:].offset,
                               ap=[rb[:].ap[0], [0, G]])
                nc.vector.tensor_tensor(
                    out=blk[:], in0=blk[:], in1=rb_b,
                    op=mybir.AluOpType.subtract)
                # rem = nx & 63 (low 6 bits of flat come from nx)
                remi = pool.tile([128, G], I32, tag="remi")
                nc.vector.tensor_scalar(
                    out=remi[:], in0=ptx[:, 1:G + 1], scalar1=63,
                    scalar2=None, op0=mybir.AluOpType.bitwise_and)
                rem = pool.tile([128, G], F32, tag="rem")
                nc.vector.tensor_copy(rem[:], remi[:])

                vals = pool.tile([128, G, 64], F32, tag="vals")
                nc.vector.memset(vals[:], 0.0)
                for g in range(G):
                    nc.gpsimd.indirect_dma_start(
                        out=vals[:, g, :], out_offset=None,
                        in_=cost[:, :],
                        in_offset=bass.IndirectOffsetOnAxis(
                            ap=blk[:, g:g + 1], axis=0),
                        bounds_check=GROWS - 1,
                        oob_is_err=False)

                # mask[p,g,c] = (io64[p,c] == rem[p,g]) ; dot with vals
                mask = pool.tile([128, G, 64], F32, tag="mask")
                nc.vector.tensor_tensor(
                    out=mask[:], in0=_bcast(io64[:], 1, G),
                    in1=_bcast(rem[:], 2, 64), op=mybir.AluOpType.is_equal)
                nc.vector.tensor_tensor(
                    out=mask[:], in0=mask[:], in1=vals[:],
                    op=mybir.AluOpType.mult)
                s1 = pool.tile([128, 1], F32, tag="s1")
                nc.vector.reduce_sum(
                    s1[:], mask[:].rearrange("p g c -> p (g c)"),
                    axis=mybir.AxisListType.X)
                s2 = pool.tile([128, 1], F32, tag="s2")
                nc.gpsimd.partition_all_reduce(
                    s2[:], s1[:], channels=128,
                    reduce_op=bass_isa.ReduceOp.add)
                nc.sync.dma_start(out=out[0:1], in_=s2[0:1, 0:1])

    nc.compile()
    return nc


def _get_gather_nc(n_iters: int = 1):
    key = ("gather", n_iters)
    if key not in _cache:
        _cache[key] = _build_gather_nc(n_iters)
    return _cache[key]


def run_gather(cost_matrix: np.ndarray, path: np.ndarray,
               n_iters: int = 1) -> np.ndarray:
    nc = _get_gather_nc(n_iters)
    cost_c = np.ascontiguousarray(cost_matrix)
    pfull = np.concatenate([path.astype(np.int32), path[:1].astype(np.int32)])
    in_maps = []
    for k in range(NCORES):
        shard = cost_c[k * (N // NCORES):(k + 1) * (N // NCORES), :]
        in_maps.append({
            "cost": shard.reshape(GROWS, 64),
            "pseg": pfull.copy(),
            "rbase": np.full((128, 1), k * GROWS, dtype=np.int32),
        })
    res = run_bass_kernel_spmd(nc, in_maps, core_ids=list(range(NCORES)))
    total = np.float32(0.0)
    for k in range(NCORES):
        total += np.asarray(res.results[k]["part"], dtype=np.float32)[0]
    return np.asarray([total], dtype=np.float32)


def kernel(solution_matrix: np.ndarray, cost_matrix: np.ndarray) -> np.ndarray:
    path = run_argmax(solution_matrix)
    return run_gather(cost_matrix, path)


if __name__ == "__main__":
    rng = np.random.default_rng(0)
    sol = rng.standard_normal((N, N), dtype=np.float32)
    cm = rng.random((N, N), dtype=np.float32)
    path = run_argmax(sol)
    want = sol.argmax(axis=0)
    print("argmax match:", np.array_equal(path, want),
          int((path != want).sum()), "mismatches")
    got = run_gather(cm, path)
    nxt = np.roll(want, -1)
    exp = cm[want, nxt].sum()
    print("cost:", got, "expected:", exp,
          "rel:", abs(got[0] - exp) / abs(exp))
